# revision 1
# baseline (speedup 1.0000x reference)
"""Trainium2 Bass kernel for nn_JointSelfAttentionLayer.

Math restructuring (both outputs are sequence-means):
  C[b]    = (1/SC) * colsum_b @ x_d[b] @ W_vd,  colsum_b[t] = sum_s softmax(logits)[s,t]/sqrt(D)
  Dout[b] = (1/(SD*sqrt(D))) * (sum_s x_c[b,s,:]) @ W_vc      (softmax rows sum to 1)
so the only heavy work is logits = x_c @ (W_qc @ W_kd^T) @ x_d^T plus a
streaming softmax column-sum. Never materializes Q, K, V_c, V_d, or A@V.

f16 hi/lo 3-pass matmuls give fp32-grade products (probed 3e-5 abs on
K=1024 N(0,1) dots) at 3x the fp32 matmul rate.
"""
import numpy as np
from contextlib import ExitStack

B, SC, SD, D = 8, 2048, 2048, 1024
P = 128
DB = D // P            # 8 d-blocks
TB = SD // P           # 16 t-blocks
SBK = SC // P          # 16 s-blocks
CH = 512
NCH = SC // CH         # 4 chunks
INV_SQRT_D = 1.0 / 32.0


def _split_excess_waits(nc, mybir, max_waits=1):
    n = 0
    ctr = [0]
    for fn in nc.m.functions:
        for bb in fn.blocks:
            out = []
            changed = False
            for inst in bb.instructions:
                si = inst.sync_info
                ws = list(si.on_wait) if (si and si.on_wait) else []
                if len(ws) > max_waits and inst.engine != mybir.EngineType.Unassigned:
                    keep = ws[:max_waits]
                    excess = ws[max_waits:]
                    for i in range(0, len(excess), max_waits):
                        chunk = excess[i:i + max_waits]
                        nop = mybir.InstNoOp(name=f"ws_{ctr[0]}", ins=[], outs=[])
                        ctr[0] += 1
                        nop.engine = inst.engine
                        nop.sync_info = mybir.SyncInfo(on_wait=chunk, on_update=[])
                        out.append(nop)
                    inst.sync_info = mybir.SyncInfo(
                        on_wait=keep, on_update=list(si.on_update or []))
                    changed = True
                    n += 1
                out.append(inst)
            if changed:
                bb.instructions = out
    return n


def _build(repeats=1):
    import concourse.bass as bass
    import concourse.tile as tile
    from concourse import mybir
    from concourse.masks import make_identity

    F32 = mybir.dt.float32
    F16 = mybir.dt.float16
    Act = mybir.ActivationFunctionType
    Alu = mybir.AluOpType
    AxX = mybir.AxisListType.X

    nc = bass.Bass("TRN2", target_bir_lowering=False, debug=False, num_devices=8)
    xc = nc.dram_tensor("x_c", [SC, D], F32, kind="ExternalInput").ap()
    xd = nc.dram_tensor("x_d", [SD, D], F32, kind="ExternalInput").ap()
    wqc = nc.dram_tensor("W_qc", [D, D], F32, kind="ExternalInput").ap()
    wvc = nc.dram_tensor("W_vc", [D, D], F32, kind="ExternalInput").ap()
    wkd = nc.dram_tensor("W_kd", [D, D], F32, kind="ExternalInput").ap()
    wvd = nc.dram_tensor("W_vd", [D, D], F32, kind="ExternalInput").ap()
    out_d = nc.dram_tensor("out", [P, 16], F32, kind="ExternalOutput").ap()

    with tile.TileContext(nc) as tc, ExitStack() as ctx:
        const = ctx.enter_context(tc.tile_pool(name="const", bufs=1))
        ident = const.tile([P, P], F32, name="ident")
        make_identity(nc, ident[:])
        cp = const.tile([P, SD], F32, name="cp")
        xsum = const.tile([P, DB], F32, name="xsum")
        out_sb = const.tile([P, 16], F32, name="out_sb")
        colsT = const.tile([P, TB], F32, name="colsT")

        for _r in range(repeats):
            nc.gpsimd.memset(cp[:], 0.0)
            nc.gpsimd.memset(xsum[:], 0.0)
            with tc.tile_pool(name=f"gtp_{_r}", bufs=1) as gtp:
                gt_h = [gtp.tile([P, SC], F16, name=f"gt_h{j}_{_r}") for j in range(DB)]
                gt_l = [gtp.tile([P, SC], F16, name=f"gt_l{j}_{_r}") for j in range(DB)]

                # ---- phase 1 + 2 under wqk scope ----
                with tc.tile_pool(name=f"wqkp_{_r}", bufs=1) as wqkp:
                    wqk_h = [wqkp.tile([P, D], F16, name=f"wqk_h{i}_{_r}") for i in range(DB)]
                    wqk_l = [wqkp.tile([P, D], F16, name=f"wqk_l{i}_{_r}") for i in range(DB)]

                    # phase 1: Wqk = W_qc @ W_kd^T in fp32
                    with tc.tile_pool(name=f"ph1_{_r}", bufs=1) as ph1, \
                         tc.tile_pool(name=f"ph1w_{_r}", bufs=2) as ph1w:
                        wqcT = [ph1.tile([P, D], F32, name=f"wqcT{k}_{_r}") for k in range(DB)]
                        wkdT = [ph1.tile([P, D], F32, name=f"wkdT{k}_{_r}") for k in range(DB)]
                        with tc.tile_pool(name=f"ph1ps_{_r}", bufs=4, space="PSUM") as ph1ps:
                            for (dram, dstT, nm) in ((wqc, wqcT, "q"), (wkd, wkdT, "k")):
                                for ih in range(2):
                                    wts = []
                                    for i4 in range(4):
                                        i = ih * 4 + i4
                                        wt = ph1w.tile([P, D], F32, name=f"w{nm}_{i}_{_r}", tag=f"w{i4}")
                                        nc.gpsimd.dma_start(wt[:], dram[i * P:(i + 1) * P, :])
                                        wts.append(wt)
                                    for k in range(DB):
                                        tp = ph1ps.tile([P, 4 * P], F32, name=f"tp{nm}_{ih}_{k}_{_r}", tag="tp")
                                        for i4 in range(4):
                                            nc.tensor.transpose(tp[:, i4 * P:(i4 + 1) * P],
                                                                wts[i4][:, k * P:(k + 1) * P], ident[:])
                                        nc.scalar.activation(
                                            dstT[k][:, ih * 4 * P:(ih + 1) * 4 * P], tp[:], Act.Copy)
                        tmp1 = ph1.tile([P, CH], F32, name=f"tmp1_{_r}")
                        with tc.tile_pool(name=f"ph1ps2_{_r}", bufs=2, space="PSUM") as ph1ps2:
                            for i in range(DB):
                                for c in range(2):
                                    pq = ph1ps2.tile([P, CH], F32, name=f"pq_{i}_{c}_{_r}", tag="pq")
                                    for k in range(DB):
                                        nc.tensor.matmul(
                                            pq[:], wqcT[k][:, i * P:(i + 1) * P],
                                            wkdT[k][:, c * CH:(c + 1) * CH],
                                            start=(k == 0), stop=(k == DB - 1))
                                    sl = slice(c * CH, (c + 1) * CH)
                                    nc.scalar.activation(wqk_h[i][:, sl], pq[:], Act.Copy)
                                    nc.vector.tensor_copy(tmp1[:], wqk_h[i][:, sl])
                                    nc.vector.tensor_sub(wqk_l[i][:, sl], pq[:], tmp1[:])

                    # phase 2: stream x_c -> GT (f16x3) + xsum
                    with tc.tile_pool(name=f"ph2_{_r}", bufs=2) as ph2, \
                         tc.tile_pool(name=f"ph2ps_{_r}", bufs=6, space="PSUM") as ps_t, \
                         tc.tile_pool(name=f"ph2ps2_{_r}", bufs=2, space="PSUM") as ps_g:
                        for c in range(NCH):
                            xh = [ph2.tile([P, CH], F16, name=f"xh{c}_{j}_{_r}", tag=f"xh{j}")
                                  for j in range(DB)]
                            xl = [ph2.tile([P, CH], F16, name=f"xl{c}_{j}_{_r}", tag=f"xl{j}")
                                  for j in range(DB)]
                            tmp = ph2.tile([P, CH], F32, name=f"tmp{c}_{_r}", tag="tmp")
                            red = ph2.tile([P, 1], F32, name=f"red{c}_{_r}", tag="red")
                            xts = []
                            for s in range(4):
                                xt = ph2.tile([P, D], F32, name=f"xt{c}_{s}_{_r}", tag=f"xt{s}")
                                nc.gpsimd.dma_start(xt[:], xc[c * CH + s * P:c * CH + (s + 1) * P, :])
                                xts.append(xt)
                            for j in range(DB):
                                tp = ps_t.tile([P, CH], F32, name=f"t2_{c}_{j}_{_r}", tag="tp")
                                for s in range(4):
                                    nc.tensor.transpose(tp[:, s * P:(s + 1) * P],
                                                        xts[s][:, j * P:(j + 1) * P], ident[:])
                                nc.scalar.activation(xh[j][:], tp[:], Act.Copy)
                                nc.vector.tensor_copy(tmp[:], xh[j][:])
                                nc.vector.tensor_sub(xl[j][:], tp[:], tmp[:])
                                nc.vector.tensor_reduce(red[:], tp[:], AxX, Alu.add)
                                nc.vector.tensor_add(xsum[:, j:j + 1], xsum[:, j:j + 1], red[:])
                            tmpg = ph2.tile([P, CH], F32, name=f"tmpg{c}_{_r}", tag="tmpg")
                            for jp in range(DB):
                                pg = ps_g.tile([P, CH], F32, name=f"pg{c}_{jp}_{_r}", tag="pg")
                                first = True
                                for (wt_, xt_) in ((wqk_h, xh), (wqk_h, xl), (wqk_l, xh)):
                                    for i in range(DB):
                                        nc.tensor.matmul(
                                            pg[:], wt_[i][:, jp * P:(jp + 1) * P], xt_[i][:],
                                            start=first, stop=False)
                                        first = False
                                sl = slice(c * CH, (c + 1) * CH)
                                nc.scalar.activation(gt_h[jp][:, sl], pg[:], Act.Copy)
                                nc.vector.tensor_copy(tmpg[:], gt_h[jp][:, sl])
                                nc.vector.tensor_sub(gt_l[jp][:, sl], pg[:], tmpg[:])

                # ---- phase 3: stream x_d -> x_dT hi/lo ----
                with tc.tile_pool(name=f"xdtp_{_r}", bufs=1) as xdtp:
                    xdt_h = [xdtp.tile([P, SD], F16, name=f"xdt_h{j}_{_r}") for j in range(DB)]
                    xdt_l = [xdtp.tile([P, SD], F16, name=f"xdt_l{j}_{_r}") for j in range(DB)]
                    with tc.tile_pool(name=f"ph3_{_r}", bufs=2) as ph3, \
                         tc.tile_pool(name=f"ph3ps_{_r}", bufs=6, space="PSUM") as ps3:
                        for c in range(NCH):
                            tmp = ph3.tile([P, CH], F32, name=f"t3m{c}_{_r}", tag="tmp")
                            xts = []
                            for s in range(4):
                                xt = ph3.tile([P, D], F32, name=f"x3t{c}_{s}_{_r}", tag=f"xt{s}")
                                nc.gpsimd.dma_start(xt[:], xd[c * CH + s * P:c * CH + (s + 1) * P, :])
                                xts.append(xt)
                            for j in range(DB):
                                tp = ps3.tile([P, CH], F32, name=f"t3_{c}_{j}_{_r}", tag="tp")
                                for s in range(4):
                                    nc.tensor.transpose(tp[:, s * P:(s + 1) * P],
                                                        xts[s][:, j * P:(j + 1) * P], ident[:])
                                csl = slice(c * CH, (c + 1) * CH)
                                nc.scalar.activation(xdt_h[j][:, csl], tp[:], Act.Copy)
                                nc.vector.tensor_copy(tmp[:], xdt_h[j][:, csl])
                                nc.vector.tensor_sub(xdt_l[j][:, csl], tp[:], tmp[:])

                    # ---- phase 4: logits + softmax colsum ----
                    with tc.tile_pool(name=f"ph4_{_r}", bufs=2) as ph4, \
                         tc.tile_pool(name=f"ph4s_{_r}", bufs=2) as ph4s, \
                         tc.tile_pool(name=f"ph4ps_{_r}", bufs=2, space="PSUM") as ph4ps:
                        for sb in range(SBK):
                            L = ph4ps.tile([P, SD], F32, name=f"L{sb}_{_r}", tag="L")
                            ssl = slice(sb * P, (sb + 1) * P)
                            for c in range(NCH):
                                tsl = slice(c * CH, (c + 1) * CH)
                                first = True
                                for (gt, xdt) in ((gt_h, xdt_h), (gt_h, xdt_l), (gt_l, xdt_h)):
                                    for j in range(DB):
                                        nc.tensor.matmul(
                                            L[:, tsl], gt[j][:, ssl], xdt[j][:, tsl],
                                            start=first, stop=False)
                                        first = False
                            mx = ph4s.tile([P, 1], F32, name=f"mx{sb}_{_r}", tag="mx")
                            nc.vector.tensor_reduce(mx[:], L[:], AxX, Alu.max)
                            negmx = ph4s.tile([P, 1], F32, name=f"negmx{sb}_{_r}", tag="negmx")
                            nc.vector.tensor_scalar_mul(negmx[:], mx[:], -1.0)
                            E = ph4.tile([P, SD], F32, name=f"E{sb}_{_r}", tag="E")
                            rs = ph4s.tile([P, 1], F32, name=f"rs{sb}_{_r}", tag="rs")
                            nc.scalar.activation(E[:], L[:], Act.Exp,
                                                 bias=negmx[:], scale=1.0, accum_out=rs[:])
                            w = ph4s.tile([P, 1], F32, name=f"w{sb}_{_r}", tag="w")
                            nc.vector.reciprocal(w[:], rs[:])
                            Et = ph4.tile([P, SD], F32, name=f"Et{sb}_{_r}", tag="Et")
                            nc.vector.tensor_scalar(Et[:], E[:], w[:], INV_SQRT_D,
                                                    Alu.mult, Alu.mult)
                            nc.vector.tensor_add(cp[:], cp[:], Et[:])

            # ---- phase 5/6: epilogues (gt/xdt pools freed) ----
            with tc.tile_pool(name=f"ph5_{_r}", bufs=2) as ph5, \
                 tc.tile_pool(name=f"ph5c_{_r}", bufs=1) as ph5c, \
                 tc.tile_pool(name=f"ph5ps_{_r}", bufs=2, space="PSUM") as ph5ps, \
                 tc.tile_pool(name=f"ph5ps2_{_r}", bufs=1, space="PSUM") as ph5ps2:
                for t in range(TB):
                    tp = ph5ps.tile([P, P], F32, name=f"cpt{t}_{_r}", tag="cpt")
                    nc.tensor.transpose(tp[:], cp[:, t * P:(t + 1) * P], ident[:])
                    nc.vector.tensor_reduce(colsT[:, t:t + 1], tp[:], AxX, Alu.add)
                pu = ph5ps2.tile([P, DB], F32, name=f"pu_{_r}")
                for t in range(TB):
                    xdn = ph5.tile([P, D], F32, name=f"xdn{t}_{_r}", tag="xdn")
                    nc.gpsimd.dma_start(xdn[:], xd[t * P:(t + 1) * P, :])
                    for j in range(DB):
                        nc.tensor.matmul(pu[:, j:j + 1], xdn[:, j * P:(j + 1) * P],
                                         colsT[:, t:t + 1],
                                         start=(j == 0 and t == 0), stop=False)
                u_sb = ph5c.tile([P, DB], F32, name=f"u_sb_{_r}")
                nc.vector.tensor_copy(u_sb[:], pu[:])
                pc = ph5ps2.tile([P, DB], F32, name=f"pc_{_r}")
                for i in range(DB):
                    wvdt = ph5.tile([P, D], F32, name=f"wvdt{i}_{_r}", tag="wvdt")
                    nc.gpsimd.dma_start(wvdt[:], wvd[i * P:(i + 1) * P, :])
                    for e in range(DB):
                        nc.tensor.matmul(pc[:, e:e + 1], wvdt[:, e * P:(e + 1) * P],
                                         u_sb[:, i:i + 1],
                                         start=(e == 0 and i == 0), stop=False)
                nc.scalar.activation(out_sb[:, 0:DB], pc[:], Act.Copy, scale=1.0 / SC)

                pd = ph5ps2.tile([P, DB], F32, name=f"pd_{_r}")
                for i in range(DB):
                    wvct = ph5.tile([P, D], F32, name=f"wvct{i}_{_r}", tag="wvct")
                    nc.gpsimd.dma_start(wvct[:], wvc[i * P:(i + 1) * P, :])
                    for e in range(DB):
                        nc.tensor.matmul(pd[:, e:e + 1], wvct[:, e * P:(e + 1) * P],
                                         xsum[:, i:i + 1],
                                         start=(e == 0 and i == 0), stop=False)
                nc.scalar.activation(out_sb[:, DB:16], pd[:], Act.Copy,
                                     scale=1.0 / (SD * 32.0))
                nc.scalar.dma_start(out_d[:], out_sb[:])

    _split_excess_waits(nc, mybir)
    return nc


def kernel(x_c, x_d, W_qc, W_vc, W_kd, W_vd):
    from concourse.bass_utils import run_bass_kernel_spmd
    nc = _build()
    in_maps = []
    for b in range(B):
        in_maps.append({
            "x_c": np.ascontiguousarray(x_c[b]),
            "x_d": np.ascontiguousarray(x_d[b]),
            "W_qc": np.asarray(W_qc), "W_vc": np.asarray(W_vc),
            "W_kd": np.asarray(W_kd), "W_vd": np.asarray(W_vd),
        })
    res = run_bass_kernel_spmd(nc, in_maps, list(range(B))).results
    C = np.empty((B, D), dtype=np.float32)
    Dout = np.empty((B, D), dtype=np.float32)
    for b in range(B):
        o = res[b]["out"]
        C[b] = o[:, :DB].T.ravel()
        Dout[b] = o[:, DB:16].T.ravel()
    return (C, Dout)



# revision 24
# speedup vs baseline: 1.9950x; 1.9950x over previous
"""Trainium2 Bass kernel for nn_JointSelfAttentionLayer.

Math restructuring (both outputs are sequence-means):
  C[b]    = (1/(SC*sqrt(D))) * w_b @ x_d[b] @ W_vd,  w_b[t] = sum_s softmax(logits)[s,t]
  Dout[b] = (1/(SD*sqrt(D))) * (sum_s x_c[b,s,:]) @ W_vc   (softmax rows sum to 1)
with logits = x_c @ Wqk @ x_d^T, Wqk = W_qc @ W_kd^T. Never materializes
Q, K, V_c, V_d, or A@V.

All heavy matmuls run single-pass f16 (measured end-to-end rel err ~2e-3,
tolerance 2e-2): logit abs error ~0.03 vs logit std 32, and softmax colsum
averaging keeps the output error small. Data layout: x_c, x_d, W_qc, W_kd
are PE-transposed once (f32, psum->f16 copies); every GEMM then contracts
along natural partition dims:
  WqcT/WkdT -> Wqk (f16) -> GT = (x_c Wqk)^T via xcT -> L = GT^T-slices @ xdT
  per s-block: rowmax (DVE) -> exp+rowsum (ACT) -> cp += E/Z (DVE fused)
  w = colsum cp (gpsimd partition reduce), uT = <xdT, w> (DVE ttr),
  C = uT@Wvd, Dout = rT@Wvc (f16 matmuls, scaled 1/65536).

SBUF pools are a stack allocator: lifetimes are nested LIFO, peak
~189 KB/partition.
"""
import numpy as np
from contextlib import ExitStack

B, SC, SD, D = 8, 2048, 2048, 1024
P = 128
DB = D // P            # 8 d-blocks
CH = 512
NCH = SC // CH         # 4 chunks of 512 rows
SBK = SC // P          # 16 s-blocks
SCALE_OUT = 1.0 / (SC * 32.0)   # 1/(SC*sqrt(D)) == 1/(SD*sqrt(D))


def _split_excess_waits(nc, mybir, max_waits=1):
    n = 0
    ctr = [0]
    for fn in nc.m.functions:
        for bb in fn.blocks:
            out = []
            changed = False
            for inst in bb.instructions:
                si = inst.sync_info
                ws = list(si.on_wait) if (si and si.on_wait) else []
                if len(ws) > max_waits and inst.engine != mybir.EngineType.Unassigned:
                    keep = ws[:max_waits]
                    excess = ws[max_waits:]
                    for i in range(0, len(excess), max_waits):
                        chunk = excess[i:i + max_waits]
                        nop = mybir.InstNoOp(name=f"ws_{ctr[0]}", ins=[], outs=[])
                        ctr[0] += 1
                        nop.engine = inst.engine
                        nop.sync_info = mybir.SyncInfo(on_wait=chunk, on_update=[])
                        out.append(nop)
                    inst.sync_info = mybir.SyncInfo(
                        on_wait=keep, on_update=list(si.on_update or []))
                    changed = True
                    n += 1
                out.append(inst)
            if changed:
                bb.instructions = out
    return n


def _build(repeats=1):
    import concourse.bass as bass
    import concourse.tile as tile
    from concourse import mybir
    from concourse.masks import make_identity

    F32 = mybir.dt.float32
    F16 = mybir.dt.float16
    Act = mybir.ActivationFunctionType
    Alu = mybir.AluOpType
    AxX = mybir.AxisListType.X
    AxC = mybir.AxisListType.C

    nc = bass.Bass("TRN2", target_bir_lowering=False, debug=False, num_devices=8)
    xc = nc.dram_tensor("x_c", [SC, D], F32, kind="ExternalInput").ap()
    xd = nc.dram_tensor("x_d", [SD, D], F32, kind="ExternalInput").ap()
    wqc = nc.dram_tensor("W_qc", [D, D], F32, kind="ExternalInput").ap()
    wvc = nc.dram_tensor("W_vc", [D, D], F32, kind="ExternalInput").ap()
    wkd = nc.dram_tensor("W_kd", [D, D], F32, kind="ExternalInput").ap()
    wvd = nc.dram_tensor("W_vd", [D, D], F32, kind="ExternalInput").ap()
    out_d = nc.dram_tensor("out", [1, 2 * D], F32, kind="ExternalOutput").ap()

    with tile.TileContext(nc) as tc, ExitStack() as ctx:
        const = ctx.enter_context(tc.tile_pool(name="const", bufs=1))
        ident = const.tile([P, P], F32, name="ident")
        make_identity(nc, ident[:])
        cp = const.tile([P, SD], F32, name="cp")          # colsum partials
        # packed smalls: cols [0:8]=r (colsum x_c), [8:16]=uacc (w @ x_d)
        smalls = const.tile([P, 2 * DB], F32, name="smalls")
        s16 = const.tile([P, 2 * DB], F16, name="s16")    # f16 of the same
        out_sb = const.tile([1, 2 * D], F32, name="out_sb")
        ones32 = const.tile([P, 1], F32, name="ones32")
        nc.gpsimd.memset(ones32[:], 1.0)

        for _r in range(repeats):
            nc.gpsimd.memset(cp[:], 0.0)
            with tc.tile_pool(name=f"gtp_{_r}", bufs=1) as gt_p, \
                 tc.tile_pool(name=f"xdT_{_r}", bufs=1) as xdT_p:
                gt = [gt_p.tile([P, SC], F16, name=f"gt{j}_{_r}")
                      for j in range(DB)]
                xdT = [xdT_p.tile([P, SD], F16, name=f"xdT{j}_{_r}")
                       for j in range(DB)]

                with tc.tile_pool(name=f"xcT_{_r}", bufs=1) as xcT_p, \
                     tc.tile_pool(name=f"wqk16_{_r}", bufs=1) as wqk16_p, \
                     tc.tile_pool(name=f"raw_{_r}", bufs=2) as raw_p, \
                     tc.tile_pool(name=f"trps_{_r}", bufs=4, space="PSUM") as tr_ps, \
                     tc.tile_pool(name=f"mmps_{_r}", bufs=4, space="PSUM") as mm_ps:
                    xcT = [xcT_p.tile([P, SC], F16, name=f"xcT{j}_{_r}")
                           for j in range(DB)]
                    wqk16 = [wqk16_p.tile([P, D], F16, name=f"wqk{i}_{_r}")
                             for i in range(DB)]

                    # ---- W_qc/W_kd: load (sync ring), transpose f32 -> f16 ----
                    with tc.tile_pool(name=f"wT_{_r}", bufs=1) as wT_p:
                        wqcT = [wT_p.tile([P, D], F16, name=f"wqcT{k}_{_r}")
                                for k in range(DB)]
                        wkdT = [wT_p.tile([P, D], F16, name=f"wkdT{k}_{_r}")
                                for k in range(DB)]
                        for (dram, dstT, nm) in ((wqc, wqcT, "q"), (wkd, wkdT, "k")):
                            for c in range(2):
                                raw = raw_p.tile([P, 4, D], F32,
                                                 name=f"w{nm}raw{c}_{_r}", tag="raw")
                                nc.sync.dma_start(
                                    raw[:],
                                    dram[c * CH:(c + 1) * CH, :].rearrange(
                                        "(a p) e -> p a e", p=P))
                                for k in range(DB):
                                    tp = tr_ps.tile([P, CH], F32,
                                                    name=f"tw{nm}{c}{k}_{_r}",
                                                    tag="tp")
                                    for a in range(4):
                                        nc.tensor.transpose(
                                            tp[:, a * P:(a + 1) * P],
                                            raw[:, a, k * P:(k + 1) * P], ident[:])
                                    nc.scalar.activation(
                                        dstT[k][:, c * CH:(c + 1) * CH], tp[:],
                                        Act.Copy)

                        # ---- x_c: load (scalar ring), transpose -> xcT f16 ----
                        for c in range(NCH):
                            raw = raw_p.tile([P, 4, D], F32,
                                             name=f"xcraw{c}_{_r}", tag="raw")
                            nc.scalar.dma_start(
                                raw[:],
                                xc[c * CH:(c + 1) * CH, :].rearrange(
                                    "(a p) e -> p a e", p=P))
                            for j in range(DB):
                                tp = tr_ps.tile([P, CH], F32,
                                                name=f"txc{c}{j}_{_r}", tag="tp")
                                for a in range(4):
                                    nc.tensor.transpose(
                                        tp[:, a * P:(a + 1) * P],
                                        raw[:, a, j * P:(j + 1) * P], ident[:])
                                nc.scalar.activation(
                                    xcT[j][:, c * CH:(c + 1) * CH], tp[:],
                                    Act.Copy)

                        # ---- Wqk = Wqc @ Wkd^T (f16 single-pass) ----
                        for i in range(DB):
                            for cc in range(2):
                                ps = mm_ps.tile([P, CH], F32,
                                                name=f"pwqk{i}{cc}_{_r}", tag="pm")
                                for k in range(DB):
                                    nc.tensor.matmul(
                                        ps[:], wqcT[k][:, i * P:(i + 1) * P],
                                        wkdT[k][:, cc * CH:(cc + 1) * CH],
                                        start=(k == 0), stop=(k == DB - 1))
                                nc.scalar.activation(
                                    wqk16[i][:, cc * CH:(cc + 1) * CH], ps[:],
                                    Act.Copy)

                    # ---- GT = (x_c @ Wqk)^T  [d2, s] ----
                    for jp in range(DB):
                        for sc in range(NCH):
                            ps = mm_ps.tile([P, CH], F32,
                                            name=f"pgt{jp}{sc}_{_r}", tag="pm")
                            for i in range(DB):
                                nc.tensor.matmul(
                                    ps[:], wqk16[i][:, jp * P:(jp + 1) * P],
                                    xcT[i][:, sc * CH:(sc + 1) * CH],
                                    start=(i == 0), stop=(i == DB - 1))
                            nc.vector.tensor_copy(
                                gt[jp][:, sc * CH:(sc + 1) * CH], ps[:])

                    # r = colsum(x_c) from xcT (f16 -> f32 reduce)
                    for j in range(DB):
                        nc.vector.tensor_reduce(
                            smalls[:, j:j + 1], xcT[j][:], AxX, Alu.add)
                    nc.vector.tensor_copy(s16[:, 0:DB], smalls[:, 0:DB])

                    # ---- x_d: load, transpose -> xdT f16 ----
                    for c in range(NCH):
                        raw = raw_p.tile([P, 4, D], F32,
                                         name=f"xdraw{c}_{_r}", tag="raw")
                        nc.scalar.dma_start(
                            raw[:],
                            xd[c * CH:(c + 1) * CH, :].rearrange(
                                "(a p) e -> p a e", p=P))
                        for j in range(DB):
                            tp = tr_ps.tile([P, CH], F32,
                                            name=f"txd{c}{j}_{_r}", tag="tp")
                            for a in range(4):
                                nc.tensor.transpose(
                                    tp[:, a * P:(a + 1) * P],
                                    raw[:, a, j * P:(j + 1) * P], ident[:])
                            nc.scalar.activation(
                                xdT[j][:, c * CH:(c + 1) * CH], tp[:], Act.Copy)

                # ---- W_vd/W_vc: load + convert f16 (gpsimd) ----
                with tc.tile_pool(name=f"wvp_{_r}", bufs=1) as wv_p:
                    wvd16 = [wv_p.tile([P, D], F16, name=f"wvd16{i}_{_r}")
                             for i in range(DB)]
                    wvc16 = [wv_p.tile([P, D], F16, name=f"wvc16{i}_{_r}")
                             for i in range(DB)]
                    with tc.tile_pool(name=f"raw2_{_r}", bufs=2) as raw2_p:
                        for (dram, dst16, nm) in ((wvd, wvd16, "vd"),
                                                  (wvc, wvc16, "vc")):
                            for c in range(2):
                                raw = raw2_p.tile([P, 4, D], F32,
                                                  name=f"w{nm}raw{c}_{_r}",
                                                  tag="raw")
                                nc.sync.dma_start(
                                    raw[:],
                                    dram[c * CH:(c + 1) * CH, :].rearrange(
                                        "(a p) e -> p a e", p=P))
                                for a in range(4):
                                    nc.gpsimd.tensor_copy(dst16[c * 4 + a][:],
                                                          raw[:, a, :])

                    # ---- epilogue broadcast tiles (late SBUF pool) ----
                    with tc.tile_pool(name=f"episb_{_r}", bufs=1) as epi_sb:
                        w16 = epi_sb.tile([1, SD], F16, name=f"w16_{_r}")
                        wB16 = epi_sb.tile([P, SD], F16, name=f"wB16_{_r}")
                        junk16 = epi_sb.tile([P, SD], F16, name=f"junk16_{_r}")
                        ones16 = epi_sb.tile([1, P], F16, name=f"ones16_{_r}")
                        nc.gpsimd.memset(ones16[:], 1.0)

                        # ---- logits + softmax colsum, per s-block ----
                        with tc.tile_pool(name=f"Lps_{_r}", bufs=2,
                                          space="PSUM") as L_ps, \
                             tc.tile_pool(name=f"E_{_r}", bufs=2) as E_p, \
                             tc.tile_pool(name=f"sm_{_r}", bufs=3) as sm_p:
                            for sb in range(SBK):
                                L = L_ps.tile([P, SD], F32, name=f"L{sb}_{_r}",
                                              tag="L")
                                for tc_ in range(NCH):
                                    tsl = slice(tc_ * CH, (tc_ + 1) * CH)
                                    for j in range(DB):
                                        nc.tensor.matmul(
                                            L[:, tsl],
                                            gt[j][:, sb * P:(sb + 1) * P],
                                            xdT[j][:, tsl],
                                            start=(j == 0), stop=(j == DB - 1))
                                # smt cols: 0=-rowmax, 1=rowsum(exp), 2=1/rowsum
                                smt = sm_p.tile([P, 4], F32, name=f"smt{sb}_{_r}",
                                                tag="smt")
                                nc.vector.tensor_reduce(smt[:, 0:1], L[:], AxX,
                                                        Alu.max, negate=True)
                                E = E_p.tile([P, SD], F32, name=f"E{sb}_{_r}",
                                             tag="E")
                                nc.scalar.activation(E[:], L[:], Act.Exp,
                                                     bias=smt[:, 0:1], scale=1.0,
                                                     accum_out=smt[:, 1:2])
                                nc.vector.reciprocal(smt[:, 2:3], smt[:, 1:2])
                                nc.vector.scalar_tensor_tensor(
                                    cp[:], E[:], smt[:, 2:3], cp[:],
                                    Alu.mult, Alu.add)

                        # ---- epilogue ----
                        # w = colsum(cp) via ones-matmul (partition reduce)
                        with tc.tile_pool(name=f"wps_{_r}", bufs=1,
                                          space="PSUM") as wps_p:
                            wps = wps_p.tile([1, SD], F32, name=f"wps_{_r}")
                            for c in range(NCH):
                                nc.tensor.matmul(
                                    wps[:, c * CH:(c + 1) * CH], ones32[:],
                                    cp[:, c * CH:(c + 1) * CH],
                                    start=True, stop=True)
                            nc.scalar.activation(w16[:], wps[:], Act.Copy)
                        # broadcast w to all partitions via ones[1,P] matmul
                        with tc.tile_pool(name=f"wbps_{_r}", bufs=2,
                                          space="PSUM") as wb_ps:
                            for c in range(NCH):
                                ps = wb_ps.tile([P, CH], F32,
                                                name=f"wb{c}_{_r}", tag="wb")
                                nc.tensor.matmul(ps[:], ones16[:],
                                                 w16[:, c * CH:(c + 1) * CH],
                                                 start=True, stop=True)
                                nc.scalar.activation(
                                    wB16[:, c * CH:(c + 1) * CH], ps[:],
                                    Act.Copy)
                        # uT[d] = sum_t xdT[d,t] * w[t]  (DVE mult + reduce)
                        for i in range(DB):
                            nc.vector.tensor_tensor(junk16[:], xdT[i][:],
                                                    wB16[:], Alu.mult)
                            nc.vector.tensor_reduce(
                                smalls[:, DB + i:DB + i + 1], junk16[:], AxX,
                                Alu.add)
                        nc.vector.tensor_copy(s16[:, DB:2 * DB],
                                              smalls[:, DB:2 * DB])

                        with tc.tile_pool(name=f"eps_{_r}", bufs=1,
                                          space="PSUM") as epi_ps:
                            pc = epi_ps.tile([1, D], F32, name=f"pc_{_r}")
                            pd = epi_ps.tile([1, D], F32, name=f"pd_{_r}")
                            for cc in range(2):
                                csl = slice(cc * CH, (cc + 1) * CH)
                                for i in range(DB):
                                    nc.tensor.matmul(
                                        pc[:, csl], s16[:, DB + i:DB + i + 1],
                                        wvd16[i][:, csl],
                                        start=(i == 0), stop=(i == DB - 1))
                                for i in range(DB):
                                    nc.tensor.matmul(
                                        pd[:, csl], s16[:, i:i + 1],
                                        wvc16[i][:, csl],
                                        start=(i == 0), stop=(i == DB - 1))
                            nc.scalar.activation(out_sb[:, 0:D], pc[:], Act.Copy,
                                                 scale=SCALE_OUT)
                            nc.scalar.activation(out_sb[:, D:2 * D], pd[:], Act.Copy,
                                                 scale=SCALE_OUT)
                        nc.sync.dma_start(out_d[:], out_sb[:])

    _split_excess_waits(nc, mybir)
    return nc


def kernel(x_c, x_d, W_qc, W_vc, W_kd, W_vd):
    from concourse.bass_utils import run_bass_kernel_spmd
    nc = _build()
    in_maps = []
    for b in range(B):
        in_maps.append({
            "x_c": np.ascontiguousarray(x_c[b]),
            "x_d": np.ascontiguousarray(x_d[b]),
            "W_qc": np.asarray(W_qc), "W_vc": np.asarray(W_vc),
            "W_kd": np.asarray(W_kd), "W_vd": np.asarray(W_vd),
        })
    res = run_bass_kernel_spmd(nc, in_maps, list(range(B))).results
    C = np.empty((B, D), dtype=np.float32)
    Dout = np.empty((B, D), dtype=np.float32)
    for b in range(B):
        o = res[b]["out"][0]
        C[b] = o[:D]
        Dout[b] = o[D:]
    return (C, Dout)


# revision 30
# speedup vs baseline: 2.3940x; 1.2000x over previous
"""Trainium2 Bass kernel for nn_JointSelfAttentionLayer.

Math restructuring (both outputs are sequence-means):
  C[b]    = (1/(SC*sqrt(D))) * w_b @ x_d[b] @ W_vd,  w_b[t] = sum_s softmax(logits)[s,t]
  Dout[b] = (1/(SD*sqrt(D))) * (sum_s x_c[b,s,:]) @ W_vc   (softmax rows sum to 1)
with logits = x_c @ Wqk @ x_d^T, Wqk = W_qc @ W_kd^T. Never materializes
Q, K, V_c, V_d, or A@V.

All heavy matmuls run single-pass f16 (measured end-to-end rel err ~2e-3,
tolerance 2e-2): logit abs error ~0.03 vs logit std 32, and softmax colsum
averaging keeps the output error small. Data layout: x_c, x_d, W_qc, W_kd
are PE-transposed once (f32, psum->f16 copies); every GEMM then contracts
along natural partition dims:
  WqcT/WkdT -> Wqk (f16) -> GT = (x_c Wqk)^T via xcT -> L = GT^T-slices @ xdT
  per s-block: rowmax (DVE) -> exp+rowsum (ACT) -> cp += E/Z (DVE fused)
  w = colsum cp (gpsimd partition reduce), uT = <xdT, w> (DVE ttr),
  C = uT@Wvd, Dout = rT@Wvc (f16 matmuls, scaled 1/65536).

SBUF pools are a stack allocator: lifetimes are nested LIFO, peak
~189 KB/partition.
"""
import numpy as np
from contextlib import ExitStack

B, SC, SD, D = 8, 2048, 2048, 1024
P = 128
DB = D // P            # 8 d-blocks
CH = 512
NCH = SC // CH         # 4 chunks of 512 rows
SBK = SC // P          # 16 s-blocks
SCALE_OUT = 1.0 / (SC * 32.0)   # 1/(SC*sqrt(D)) == 1/(SD*sqrt(D))


def _split_excess_waits(nc, mybir, max_waits=1):
    n = 0
    ctr = [0]
    for fn in nc.m.functions:
        for bb in fn.blocks:
            out = []
            changed = False
            for inst in bb.instructions:
                si = inst.sync_info
                ws = list(si.on_wait) if (si and si.on_wait) else []
                if len(ws) > max_waits and inst.engine != mybir.EngineType.Unassigned:
                    keep = ws[:max_waits]
                    excess = ws[max_waits:]
                    for i in range(0, len(excess), max_waits):
                        chunk = excess[i:i + max_waits]
                        nop = mybir.InstNoOp(name=f"ws_{ctr[0]}", ins=[], outs=[])
                        ctr[0] += 1
                        nop.engine = inst.engine
                        nop.sync_info = mybir.SyncInfo(on_wait=chunk, on_update=[])
                        out.append(nop)
                    inst.sync_info = mybir.SyncInfo(
                        on_wait=keep, on_update=list(si.on_update or []))
                    changed = True
                    n += 1
                out.append(inst)
            if changed:
                bb.instructions = out
    return n


def _build(repeats=1):
    import concourse.bass as bass
    import concourse.tile as tile
    from concourse import mybir
    from concourse.masks import make_identity

    F32 = mybir.dt.float32
    F16 = mybir.dt.float16
    Act = mybir.ActivationFunctionType
    Alu = mybir.AluOpType
    AxX = mybir.AxisListType.X
    AxC = mybir.AxisListType.C

    nc = bass.Bass("TRN2", target_bir_lowering=False, debug=False, num_devices=8)
    xc = nc.dram_tensor("x_c", [SC, D], F32, kind="ExternalInput").ap()
    xd = nc.dram_tensor("x_d", [SD, D], F32, kind="ExternalInput").ap()
    wqc = nc.dram_tensor("W_qc", [D, D], F32, kind="ExternalInput").ap()
    wvc = nc.dram_tensor("W_vc", [D, D], F32, kind="ExternalInput").ap()
    wkd = nc.dram_tensor("W_kd", [D, D], F32, kind="ExternalInput").ap()
    wvd = nc.dram_tensor("W_vd", [D, D], F32, kind="ExternalInput").ap()
    out_d = nc.dram_tensor("out", [1, 2 * D], F32, kind="ExternalOutput").ap()

    with tile.TileContext(nc) as tc, ExitStack() as ctx:
        const = ctx.enter_context(tc.tile_pool(name="const", bufs=1))
        ident = const.tile([P, P], F32, name="ident")
        make_identity(nc, ident[:])
        cp = const.tile([P, SD], F32, name="cp")          # colsum partials
        # packed smalls: cols [0:8]=r (colsum x_c), [8:16]=uacc (w @ x_d)
        smalls = const.tile([P, 2 * DB], F32, name="smalls")
        s16 = const.tile([P, 2 * DB], F16, name="s16")    # f16 of the same
        out_sb = const.tile([1, 2 * D], F32, name="out_sb")
        ones32 = const.tile([P, 1], F32, name="ones32")
        nc.gpsimd.memset(ones32[:], 1.0)

        for _r in range(repeats):
            nc.gpsimd.memset(cp[:], 0.0)
            with tc.tile_pool(name=f"gtp_{_r}", bufs=1) as gt_p, \
                 tc.tile_pool(name=f"xdT_{_r}", bufs=1) as xdT_p:
                gt = [gt_p.tile([P, SC], F16, name=f"gt{j}_{_r}")
                      for j in range(DB)]
                xdT = [xdT_p.tile([P, SD], F16, name=f"xdT{j}_{_r}")
                       for j in range(DB)]

                with tc.tile_pool(name=f"xcT_{_r}", bufs=1) as xcT_p, \
                     tc.tile_pool(name=f"wqk16_{_r}", bufs=1) as wqk16_p, \
                     tc.tile_pool(name=f"raw_{_r}", bufs=2) as raw_p, \
                     tc.tile_pool(name=f"trps_{_r}", bufs=4, space="PSUM") as tr_ps, \
                     tc.tile_pool(name=f"mmps_{_r}", bufs=4, space="PSUM") as mm_ps:
                    xcT = [xcT_p.tile([P, SC], F16, name=f"xcT{j}_{_r}")
                           for j in range(DB)]
                    wqk16 = [wqk16_p.tile([P, D], F16, name=f"wqk{i}_{_r}")
                             for i in range(DB)]

                    # ---- W_qc/W_kd: load (sync ring), transpose f32 -> f16 ----
                    with tc.tile_pool(name=f"wT_{_r}", bufs=1) as wT_p:
                        wqcT = [wT_p.tile([P, D], F16, name=f"wqcT{k}_{_r}")
                                for k in range(DB)]
                        wkdT = [wT_p.tile([P, D], F16, name=f"wkdT{k}_{_r}")
                                for k in range(DB)]
                        for (dram, dstT, nm) in ((wqc, wqcT, "q"), (wkd, wkdT, "k")):
                            for c in range(2):
                                raw = raw_p.tile([P, 4, D], F32,
                                                 name=f"w{nm}raw{c}_{_r}", tag="raw")
                                nc.sync.dma_start(
                                    raw[:],
                                    dram[c * CH:(c + 1) * CH, :].rearrange(
                                        "(a p) e -> p a e", p=P))
                                for k in range(DB):
                                    tp = tr_ps.tile([P, CH], F32,
                                                    name=f"tw{nm}{c}{k}_{_r}",
                                                    tag="tp")
                                    for a in range(4):
                                        nc.tensor.transpose(
                                            tp[:, a * P:(a + 1) * P],
                                            raw[:, a, k * P:(k + 1) * P], ident[:])
                                    nc.scalar.activation(
                                        dstT[k][:, c * CH:(c + 1) * CH], tp[:],
                                        Act.Copy)

                        # ---- x_c: load (scalar ring), transpose -> xcT f16 ----
                        for c in range(NCH):
                            raw = raw_p.tile([P, 4, D], F32,
                                             name=f"xcraw{c}_{_r}", tag="raw")
                            nc.scalar.dma_start(
                                raw[:],
                                xc[c * CH:(c + 1) * CH, :].rearrange(
                                    "(a p) e -> p a e", p=P))
                            for j in range(DB):
                                tp = tr_ps.tile([P, CH], F32,
                                                name=f"txc{c}{j}_{_r}", tag="tp")
                                for a in range(4):
                                    nc.tensor.transpose(
                                        tp[:, a * P:(a + 1) * P],
                                        raw[:, a, j * P:(j + 1) * P], ident[:])
                                nc.scalar.activation(
                                    xcT[j][:, c * CH:(c + 1) * CH], tp[:],
                                    Act.Copy)

                        # ---- Wqk = Wqc @ Wkd^T (f16 single-pass) ----
                        # k-outer: lhsT loaded once per (i, k), reused 2x
                        for i in range(DB):
                            pss = [mm_ps.tile([P, CH], F32,
                                              name=f"pwqk{i}{cc}_{_r}", tag="pm")
                                   for cc in range(2)]
                            for k in range(DB):
                                for cc in range(2):
                                    nc.tensor.matmul(
                                        pss[cc][:], wqcT[k][:, i * P:(i + 1) * P],
                                        wkdT[k][:, cc * CH:(cc + 1) * CH],
                                        start=(k == 0), stop=(k == DB - 1))
                            for cc in range(2):
                                nc.scalar.activation(
                                    wqk16[i][:, cc * CH:(cc + 1) * CH],
                                    pss[cc][:], Act.Copy)

                    # ---- GT = (x_c @ Wqk)^T  [d2, s] ----
                    # i-outer: lhsT loaded once per (jp, i), reused 4x
                    for jp in range(DB):
                        pss = [mm_ps.tile([P, CH], F32,
                                          name=f"pgt{jp}{sc}_{_r}", tag="pm")
                               for sc in range(NCH)]
                        for i in range(DB):
                            for sc in range(NCH):
                                nc.tensor.matmul(
                                    pss[sc][:], wqk16[i][:, jp * P:(jp + 1) * P],
                                    xcT[i][:, sc * CH:(sc + 1) * CH],
                                    start=(i == 0), stop=(i == DB - 1))
                        for sc in range(NCH):
                            nc.vector.tensor_copy(
                                gt[jp][:, sc * CH:(sc + 1) * CH], pss[sc][:])

                    # r = colsum(x_c) from xcT (f16 -> f32 reduce)
                    for j in range(DB):
                        nc.vector.tensor_reduce(
                            smalls[:, j:j + 1], xcT[j][:], AxX, Alu.add)
                    nc.vector.tensor_copy(s16[:, 0:DB], smalls[:, 0:DB])

                    # ---- x_d: load, transpose -> xdT f16 ----
                    for c in range(NCH):
                        raw = raw_p.tile([P, 4, D], F32,
                                         name=f"xdraw{c}_{_r}", tag="raw")
                        nc.scalar.dma_start(
                            raw[:],
                            xd[c * CH:(c + 1) * CH, :].rearrange(
                                "(a p) e -> p a e", p=P))
                        for j in range(DB):
                            tp = tr_ps.tile([P, CH], F32,
                                            name=f"txd{c}{j}_{_r}", tag="tp")
                            for a in range(4):
                                nc.tensor.transpose(
                                    tp[:, a * P:(a + 1) * P],
                                    raw[:, a, j * P:(j + 1) * P], ident[:])
                            nc.scalar.activation(
                                xdT[j][:, c * CH:(c + 1) * CH], tp[:], Act.Copy)

                # ---- W_vd/W_vc: casting DMA load (f32 DRAM -> f16 SBUF) ----
                with tc.tile_pool(name=f"wvp_{_r}", bufs=1) as wv_p:
                    wvdt = [wv_p.tile([P, 4, D], F16, name=f"wvd16{c}_{_r}")
                            for c in range(2)]
                    wvct = [wv_p.tile([P, 4, D], F16, name=f"wvc16{c}_{_r}")
                            for c in range(2)]
                    for (dram, dst, nm) in ((wvd, wvdt, "vd"), (wvc, wvct, "vc")):
                        for c in range(2):
                            nc.gpsimd.dma_start(
                                dst[c][:],
                                dram[c * CH:(c + 1) * CH, :].rearrange(
                                    "(a p) e -> p a e", p=P))
                    wvd16 = [wvdt[i // 4][:, i % 4, :] for i in range(DB)]
                    wvc16 = [wvct[i // 4][:, i % 4, :] for i in range(DB)]

                    # ---- epilogue broadcast tiles (late SBUF pool) ----
                    with tc.tile_pool(name=f"episb_{_r}", bufs=1) as epi_sb:
                        w16 = epi_sb.tile([1, SD], F16, name=f"w16_{_r}")
                        wB16 = epi_sb.tile([P, SD], F16, name=f"wB16_{_r}")
                        junk16 = [epi_sb.tile([P, SD], F16,
                                              name=f"junk16{k}_{_r}")
                                  for k in range(2)]
                        ones16 = epi_sb.tile([1, P], F16, name=f"ones16_{_r}")
                        nc.gpsimd.memset(ones16[:], 1.0)

                        # ---- logits + softmax colsum, per s-block ----
                        with tc.tile_pool(name=f"Lps_{_r}", bufs=2,
                                          space="PSUM") as L_ps, \
                             tc.tile_pool(name=f"E_{_r}", bufs=2) as E_p, \
                             tc.tile_pool(name=f"sm_{_r}", bufs=3) as sm_p:
                            for sb in range(SBK):
                                L = L_ps.tile([P, SD], F32, name=f"L{sb}_{_r}",
                                              tag="L")
                                # j-outer: lhsT loaded once per j, reused 4x
                                for j in range(DB):
                                    for tc_ in range(NCH):
                                        tsl = slice(tc_ * CH, (tc_ + 1) * CH)
                                        nc.tensor.matmul(
                                            L[:, tsl],
                                            gt[j][:, sb * P:(sb + 1) * P],
                                            xdT[j][:, tsl],
                                            start=(j == 0), stop=(j == DB - 1))
                                # smt cols: 0=-rowmax, 1=rowsum(exp), 2=1/rowsum
                                smt = sm_p.tile([P, 4], F32, name=f"smt{sb}_{_r}",
                                                tag="smt")
                                nc.vector.tensor_reduce(smt[:, 0:1], L[:], AxX,
                                                        Alu.max, negate=True)
                                E = E_p.tile([P, SD], F32, name=f"E{sb}_{_r}",
                                             tag="E")
                                nc.scalar.activation(E[:], L[:], Act.Exp,
                                                     bias=smt[:, 0:1], scale=1.0,
                                                     accum_out=smt[:, 1:2])
                                nc.vector.reciprocal(smt[:, 2:3], smt[:, 1:2])
                                nc.vector.scalar_tensor_tensor(
                                    cp[:], E[:], smt[:, 2:3], cp[:],
                                    Alu.mult, Alu.add)

                        # ---- epilogue ----
                        # w = colsum(cp) via ones-matmul (partition reduce)
                        with tc.tile_pool(name=f"wps_{_r}", bufs=1,
                                          space="PSUM") as wps_p:
                            wps = wps_p.tile([1, SD], F32, name=f"wps_{_r}")
                            for c in range(NCH):
                                nc.tensor.matmul(
                                    wps[:, c * CH:(c + 1) * CH], ones32[:],
                                    cp[:, c * CH:(c + 1) * CH],
                                    start=True, stop=True)
                            nc.scalar.activation(w16[:], wps[:], Act.Copy)
                        # broadcast w to all partitions via ones[1,P] matmul
                        with tc.tile_pool(name=f"wbps_{_r}", bufs=2,
                                          space="PSUM") as wb_ps:
                            for c in range(NCH):
                                ps = wb_ps.tile([P, CH], F32,
                                                name=f"wb{c}_{_r}", tag="wb")
                                nc.tensor.matmul(ps[:], ones16[:],
                                                 w16[:, c * CH:(c + 1) * CH],
                                                 start=True, stop=True)
                                nc.scalar.activation(
                                    wB16[:, c * CH:(c + 1) * CH], ps[:],
                                    Act.Copy)
                        # uT[d] = sum_t xdT[d,t] * w[t]  (DVE mult + reduce)
                        for i in range(DB):
                            jk = junk16[i % 2]
                            nc.vector.tensor_tensor(jk[:], xdT[i][:],
                                                    wB16[:], Alu.mult)
                            nc.vector.tensor_reduce(
                                smalls[:, DB + i:DB + i + 1], jk[:], AxX,
                                Alu.add)
                        nc.vector.tensor_copy(s16[:, DB:2 * DB],
                                              smalls[:, DB:2 * DB])

                        with tc.tile_pool(name=f"eps_{_r}", bufs=1,
                                          space="PSUM") as epi_ps:
                            pc = epi_ps.tile([1, D], F32, name=f"pc_{_r}")
                            pd = epi_ps.tile([1, D], F32, name=f"pd_{_r}")
                            for cc in range(2):
                                csl = slice(cc * CH, (cc + 1) * CH)
                                for i in range(DB):
                                    nc.tensor.matmul(
                                        pc[:, csl], s16[:, DB + i:DB + i + 1],
                                        wvd16[i][:, csl],
                                        start=(i == 0), stop=(i == DB - 1))
                                for i in range(DB):
                                    nc.tensor.matmul(
                                        pd[:, csl], s16[:, i:i + 1],
                                        wvc16[i][:, csl],
                                        start=(i == 0), stop=(i == DB - 1))
                            nc.scalar.activation(out_sb[:, 0:D], pc[:], Act.Copy,
                                                 scale=SCALE_OUT)
                            nc.scalar.activation(out_sb[:, D:2 * D], pd[:], Act.Copy,
                                                 scale=SCALE_OUT)
                        nc.sync.dma_start(out_d[:], out_sb[:])

    _split_excess_waits(nc, mybir)
    return nc


def kernel(x_c, x_d, W_qc, W_vc, W_kd, W_vd):
    from concourse.bass_utils import run_bass_kernel_spmd
    nc = _build()
    in_maps = []
    for b in range(B):
        in_maps.append({
            "x_c": np.ascontiguousarray(x_c[b]),
            "x_d": np.ascontiguousarray(x_d[b]),
            "W_qc": np.asarray(W_qc), "W_vc": np.asarray(W_vc),
            "W_kd": np.asarray(W_kd), "W_vd": np.asarray(W_vd),
        })
    res = run_bass_kernel_spmd(nc, in_maps, list(range(B))).results
    C = np.empty((B, D), dtype=np.float32)
    Dout = np.empty((B, D), dtype=np.float32)
    for b in range(B):
        o = res[b]["out"][0]
        C[b] = o[:D]
        Dout[b] = o[D:]
    return (C, Dout)
